# revision 3
# baseline (speedup 1.0000x reference)
"""Trainium2 Bass kernel for nn_Cropper: 100 bilinear 100x100 crops per image,
8 images data-parallel across 8 NeuronCores.

v2 pipeline (c4 image, gather-then-blend, PE-upcast):
  - Host ships image as [x, y, c4] bf16 (4th channel zero pad), DECLARED f32
    ([1, W*H*2] f32; one f32 = a (c0,c1) or (c2,c3) bf16 pair). Column per x
    = 2048 f32.
  - dma_gather: descriptor per (box, u, j) fetches the 512-row y-window of
    column x_u(j): ELEM=1024 f32 (4096 B), STEP=64 f32, yb multiple of 32
    clamped to 512. G[j, g, u, 1024] f32.
  - ap_gather (f32 d=2 = one y-row c4): idx per (g, u, t, i) selects the
    2 vertical taps -> Hv[j, g, u, t, i, c4pair]. Table = G directly
    (no upcast); 1600 idx per GB=4 group.
  - DVE horizontal blend on bf16 views with per-partition wx(j):
    h[j, t, i, c4] = Hv[u0]*(1-wx) + Hv[u1]*wx  (2 ops, 800 elems, 2x mode).
  - PE transposes h[:, t, :, c] [j,i]->[i,j] per (t,c) in bf16, accumulating
    into f32 PSUM (free upcast). ACT copies PSUM->SBUF.
  - DVE vertical blend with per-partition wy(i): o = H0 + (H1-H0)*wy.
  - DMA writes [i, c, j] to out[m, c, i, j].
"""
import numpy as np
import ml_dtypes
from contextlib import ExitStack

B, NBOX, C, H, W = 8, 100, 3, 1024, 1024
S = 100
C4 = 4              # padded channels
COL = H * 2         # f32 elems per image column (H rows x c4 bf16 = 2 f32)
ELEM = 1024         # dma_gather element (f32): 512 rows x 2 f32
ROWS = ELEM // 2    # 512 y rows per window
STEP = 64           # idx granularity in f32 elems (256 B)
YB_ALIGN = 32
YB_MAX = H - ROWS   # 512
NIDX = 2 * 128      # dma_gather descriptors per box (u, j)
NAG = 2 * 2 * S     # ap_gather indices per box (u, t, i)
NPAD = 256          # f32 pad at end of image buffer
GB = 4              # boxes per dma_gather / ap_gather instruction

_CACHE = {}


def _box_geometry(boxes_b):
    fb = boxes_b.astype(np.float32)
    x0 = np.floor(fb[:, 0] * np.float32(W))
    y0 = np.floor(fb[:, 1] * np.float32(H))
    w0 = np.maximum(np.floor(fb[:, 2] * np.float32(W)), np.float32(1.0))
    h0 = np.maximum(np.floor(fb[:, 3] * np.float32(H)), np.float32(1.0))
    grid = (np.arange(S, dtype=np.float32) + np.float32(0.5)) / np.float32(S)
    sy = np.clip(grid[None, :] * h0[:, None] - np.float32(0.5),
                 np.float32(0.0), (h0 - np.float32(1.0))[:, None]) + y0[:, None]
    sx = np.clip(grid[None, :] * w0[:, None] - np.float32(0.5),
                 np.float32(0.0), (w0 - np.float32(1.0))[:, None]) + x0[:, None]
    yf = np.floor(sy)
    xf = np.floor(sx)
    wy = (sy - yf).astype(np.float32)
    wx = (sx - xf).astype(np.float32)
    y0i = np.clip(yf, 0, H - 1).astype(np.int64)
    y1i = np.clip(yf + 1, 0, H - 1).astype(np.int64)
    x0i = np.clip(xf, 0, W - 1).astype(np.int64)
    x1i = np.clip(xf + 1, 0, W - 1).astype(np.int64)
    return wy, wx, y0i, y1i, x0i, x1i


def _wrap16(vals_2d, dtype):
    """[nblk, n] -> [128, nblk*ceil(n/16)]; idx i at [i%16, i//16] per block."""
    nb, n = vals_2d.shape
    sw = (n + 15) // 16
    w = np.zeros((nb, 16, sw), dtype=dtype)
    idx = np.arange(n)
    w[:, idx % 16, idx // 16] = vals_2d
    w = w.transpose(1, 0, 2).reshape(16, nb * sw)
    return np.tile(w, (8, 1))


GIW = (GB * NIDX + 15) // 16    # wrapped gidx cols per GB group
AGW = (GB * NAG + 15) // 16     # wrapped agidx cols per GB group


def _prep_core(image_b, boxes_b):
    """image_b [C,H,W] f32, boxes_b [NBOX,4] f32 -> device input dict."""
    wy, wx, y0i, y1i, x0i, x1i = _box_geometry(boxes_b)

    yb = np.minimum((y0i.min(axis=1) // YB_ALIGN) * YB_ALIGN, YB_MAX)  # [NBOX]
    assert (y1i.max(axis=1) - yb).max() < ROWS
    assert (y0i.min(axis=1) - yb).min() >= 0

    # dma_gather idx: n = (b%GB)*NIDX + u*128 + p ; p=j
    xtap = np.stack([x0i, x1i], axis=1)                  # [NBOX, 2(u), S]
    start = xtap * COL + (yb * 2)[:, None, None]         # [NBOX, 2, S] (f32)
    assert (start % STEP == 0).all()
    gidx = start // STEP
    assert gidx.max() < 32768 and gidx.min() >= 0
    full = np.zeros((NBOX, 2, 128), dtype=np.int16)
    full[:, :, :S] = gidx.astype(np.int16)
    gidx_all = _wrap16(full.reshape(NBOX // GB, GB * NIDX), np.int16)

    # ap_gather idx over G viewed [128, GB*2*ROWS, 2]:
    # slot for (b%GB, u, t, i) = ((b%GB)*2 + u)*ROWS + (ytap(t,i) - yb)
    ytap = np.stack([y0i, y1i], axis=1)                  # [NBOX, 2(t), S]
    yrel = ytap - yb[:, None, None]                      # [NBOX, 2, S]
    assert yrel.min() >= 0 and yrel.max() < ROWS
    # per box: idx order (u, t, i)
    agv = np.broadcast_to(yrel[:, None, :, :], (NBOX, 2, 2, S)).copy()
    agv += (np.arange(2) * ROWS)[None, :, None, None]    # u offset
    agv = agv.reshape(NBOX // GB, GB, 2 * 2 * S) \
        + (np.arange(GB) * 2 * ROWS)[None, :, None]
    agidx_all = _wrap16(agv.reshape(NBOX // GB, GB * NAG).astype(np.int16),
                        np.int16)

    m1wx = np.zeros((128, NBOX), dtype=np.float32)
    wxT = np.zeros((128, NBOX), dtype=np.float32)
    m1wx[:S] = (np.float32(1.0) - wx).T
    wxT[:S] = wx.T
    wyT = np.zeros((128, NBOX), dtype=np.float32)
    wyT[:S] = wy.T

    # image [x, y, c4] bf16, viewed f32: [W*H*2 + NPAD]
    imgT = np.ascontiguousarray(image_b.transpose(2, 1, 0))  # [W, H, C] f32
    imgc4 = np.zeros((W, H, C4), dtype=ml_dtypes.bfloat16)
    imgc4[:, :, :C] = imgT.astype(ml_dtypes.bfloat16)
    img_pad = np.zeros((W * H * 2 + NPAD,), dtype=np.float32)
    img_pad[:W * H * 2] = imgc4.reshape(-1).view(np.float32)

    return {
        "img": img_pad.reshape(1, -1),
        "gidx": gidx_all,
        "agidx": agidx_all,
        "m1wx": m1wx,
        "wxT": wxT,
        "wyT": wyT,
    }


def _build_program():
    import concourse.bass as bass
    import concourse.tile as tile
    from concourse import bacc, mybir
    from concourse.masks import make_identity

    bf16 = mybir.dt.bfloat16
    f32 = mybir.dt.float32
    i16 = mybir.dt.int16
    Alu = mybir.AluOpType

    nc = bacc.Bacc("TRN2", target_bir_lowering=False, debug=False,
                   enable_asserts=False, num_devices=8)
    img_d = nc.dram_tensor("img", [1, W * H * 2 + NPAD], f32,
                           kind="ExternalInput")
    gidx_d = nc.dram_tensor("gidx", [128, (NBOX // GB) * GIW], i16,
                            kind="ExternalInput")
    agidx_d = nc.dram_tensor("agidx", [128, (NBOX // GB) * AGW], i16,
                             kind="ExternalInput")
    m1wx_d = nc.dram_tensor("m1wx", [128, NBOX], f32, kind="ExternalInput")
    wxT_d = nc.dram_tensor("wxT", [128, NBOX], f32, kind="ExternalInput")
    wyT_d = nc.dram_tensor("wyT", [128, NBOX], f32, kind="ExternalInput")
    out_d = nc.dram_tensor("out", [NBOX, C, S, S], f32, kind="ExternalOutput")

    with tile.TileContext(nc) as tc, ExitStack() as ctx:
        const = ctx.enter_context(tc.tile_pool(name="const", bufs=1))
        gidx_s = const.tile([128, (NBOX // GB) * GIW], i16)
        nc.sync.dma_start(gidx_s[:], gidx_d.ap())
        agidx_s = const.tile([128, (NBOX // GB) * AGW], i16)
        nc.sync.dma_start(agidx_s[:], agidx_d.ap())
        m1wx_s = const.tile([128, NBOX], f32)
        nc.sync.dma_start(m1wx_s[:], m1wx_d.ap())
        wxT_s = const.tile([128, NBOX], f32)
        nc.sync.dma_start(wxT_s[:], wxT_d.ap())
        wyT_s = const.tile([128, NBOX], f32)
        nc.sync.dma_start(wyT_s[:], wyT_d.ap())
        ident = const.tile([128, 128], bf16)
        make_identity(nc, ident[:])

        nrow = (W * H * 2 + NPAD - ELEM) // STEP
        in_view = bass.AP(img_d.ap().tensor, 0, [[STEP, nrow], [1, ELEM]])

        gpool = ctx.enter_context(tc.tile_pool(name="g", bufs=2))
        vpool = ctx.enter_context(tc.tile_pool(name="v", bufs=2))
        hpool = ctx.enter_context(tc.tile_pool(name="h", bufs=4))
        otpool = ctx.enter_context(tc.tile_pool(name="ot", bufs=3))
        opool = ctx.enter_context(tc.tile_pool(name="o", bufs=4))
        pspool = ctx.enter_context(tc.tile_pool(name="ps", bufs=2,
                                                space="PSUM"))

        import os as _os
        _reps = int(_os.environ.get("BASS_CROP_REPS", "1"))
        G_cur = None
        Hv_cur = None
        for m in [mm for _r in range(_reps) for mm in range(NBOX)]:
            bg, bo = m // GB, m % GB
            if bo == 0:
                G_cur = gpool.tile([128, GB, 2, ELEM], f32, tag="G")
                nc.gpsimd.dma_gather(
                    out_ap=G_cur[:].rearrange("p b u e -> p (b u) e"),
                    in_ap=in_view,
                    idxs_ap=gidx_s[:, bg * GIW:(bg + 1) * GIW],
                    num_idxs=GB * NIDX, num_idxs_reg=GB * NIDX,
                    elem_size=ELEM, elem_step=STEP,
                )
                # vertical-tap select straight from the bf16 window table
                Hv_cur = vpool.tile([128, GB, 2, 2, S, 2], f32, tag="Hv")
                nc.gpsimd.ap_gather(
                    out_ap=Hv_cur[:].rearrange("p b u t i c -> p (b u t i) c"),
                    in_ap=G_cur[:].rearrange("p b u e -> p (b u e)").rearrange(
                        "p (y c) -> p y c", c=2),
                    idxs_ap=agidx_s[:, bg * AGW:(bg + 1) * AGW],
                    channels=128, num_elems=GB * 2 * ROWS, d=2,
                    num_idxs=GB * NAG,
                )

            # horizontal blend (bf16 views, per-partition wx(j)):
            # h[j, t, i, c4] = Hv[u0]*(1-wx) + Hv[u1]*wx
            v_bf = Hv_cur[:].bitcast(bf16)   # [128, GB, 2, 2, S, 4]
            t = hpool.tile([128, 2, S, C4], bf16, tag="t")
            nc.vector.tensor_scalar(
                out=t[:], in0=v_bf[:, bo, 0],
                scalar1=m1wx_s[:, m:m + 1], scalar2=None, op0=Alu.mult)
            h = hpool.tile([128, 2, S, C4], bf16, tag="h")
            nc.vector.scalar_tensor_tensor(
                out=h[:], in0=v_bf[:, bo, 1],
                scalar=wxT_s[:, m:m + 1], in1=t[:],
                op0=Alu.mult, op1=Alu.add)

            # transpose [j, i] -> [i, j] per (t, c) via PE; PSUM is f32
            ps = pspool.tile([S, 2, C, 128], bf16, tag="ps")
            for tt in range(2):
                for c in range(C):
                    nc.tensor.transpose(
                        out=ps[:, tt, c, :], in_=h[:, tt, :, c],
                        identity=ident[:])
            HvT = otpool.tile([S, 2, C, S], f32, tag="HvT")
            nc.scalar.copy(out=HvT[:], in_=ps[:, :, :, :S])

            # final vertical blend (f32), wy per-partition (i):
            dv = opool.tile([S, C, S], f32, tag="dv")
            nc.vector.tensor_tensor(
                out=dv[:], in0=HvT[:, 1], in1=HvT[:, 0],
                op=Alu.subtract)
            o = opool.tile([S, C, S], f32, tag="o")
            nc.vector.scalar_tensor_tensor(
                out=o[:], in0=dv[:], scalar=wyT_s[:S, m:m + 1],
                in1=HvT[:, 0], op0=Alu.mult, op1=Alu.add)

            dst = out_d.ap()[m].transpose([1, 0, 2])  # [S(i), C, S(j)]
            nc.sync.dma_start(dst, o[:])

    nc.compile()
    return nc


def kernel(images: np.ndarray, boxes: np.ndarray) -> np.ndarray:
    images = np.asarray(images, dtype=np.float32)
    boxes = np.asarray(boxes, dtype=np.float32)
    assert images.shape == (B, C, H, W) and boxes.shape == (B, NBOX, 4)

    if "nc" not in _CACHE:
        _CACHE["nc"] = _build_program()
    nc = _CACHE["nc"]

    in_maps = [_prep_core(images[b], boxes[b]) for b in range(B)]

    from concourse.bass_utils import run_bass_kernel_spmd
    res = run_bass_kernel_spmd(nc, in_maps, core_ids=list(range(B)))
    out = np.stack([res.results[b]["out"] for b in range(B)], axis=0)
    return out.reshape(B * NBOX, C, S, S)


if __name__ == "__main__":
    import reference
    inputs = {k: np.asarray(v) for k, v in reference.setup_inputs().items()}
    got = kernel(**inputs)
    expected = np.asarray(reference.reference(**inputs))
    err = np.abs(got - expected)
    denom = np.abs(expected).max()
    print("max abs err:", err.max(), " rel:", err.max() / denom)
